# revision 58
# baseline (speedup 1.0000x reference)
"""Pairwise KL divergence kernel for Trainium2, SPMD across 8 NeuronCores.

out[n, m] = sum_d a[n,d]*(log a[n,d] - log b[m,d])
          = ent[n] - (a @ log(b)^T)[n, m],  ent = rowsum(a * log a)

Sharding: a (and output rows) split 8 ways; b replicated.
Per core: a_shard (1024, 64), b (8192, 64) -> out_shard (1024, 8192).

Design (v3): the device does the O(N*M*D) GEMM and the O(N*M)
quantize+store — everything O((N+M)*D) is host-side operand prep
(log(b), entropy/bias, layout, fp16 cast), per the sharding hint
("a local GEMM against log(b)^T").

  - lb: log(b)^T fp16, m-tiles 0..31 on partitions 0..63 (d axis),
    m-tiles 32..63 on partitions 64..127, so the two K=64 matmuls of a
    group run CONCURRENTLY on disjoint PE row halves (full PE rate).
  - aT: -a_shard^T fp16 duplicated on both partition halves (negation
    makes psum = -cross so the evac is a pure mult+add).
  - qb: [inv | B] fp32; B[p,t] = ent*inv + cst encodes the entropy term
    and the int8 affine code in the evac's per-partition scalars.
  - main loop: 8 n-tiles x 8 groups; group g -> psum slot g%4 (2 banks);
    fused evac (tensor_scalar / activation: q = rne(psum*inv + B), int8,
    FD=1024) on a statically-assigned engine (~31:33 DVE:ACT balance;
    static slot->engine keeps each matmul's psum-free wait on one
    semaphore); stores every 4 groups, finer on the last tile.
  - int8 output (saturating RNE on HW) halves store traffic; the quant
    range [lo, hi] is a rigorous host bound (Cauchy-Schwarz + column
    extrema), so clipping never triggers; ~0.4% rel err vs the 2e-2 gate.
  - Evac is the roofline: PSUM fp32 reads run at 1 elem/cycle/partition
    (DVE 0.96 GHz + ACT 1.2 GHz), ~42 us/core for the 8.4M elems.
"""

import numpy as np

N, M, D = 8192, 8192, 64
NCORES = 8
NSHARD = N // NCORES          # 1024 rows of a per core
NT = NSHARD // 128            # 8 n-tiles per core
K2 = M // 128 // 2            # 32 m-tile pairs (h0: tiles 0..31, h1: 32..63)
G = 8                         # groups per n-tile; each = 1024 out cols

OUT_MODE = "i8"               # "i8" | "f16"

_CACHE = {}


def _build(out_mode):
    from contextlib import ExitStack

    import concourse.bacc as bacc_mod
    import concourse.bass as bass
    import concourse.mybir as mybir
    import concourse.tile as tile

    FP32 = mybir.dt.float32
    F16 = mybir.dt.float16
    I8 = mybir.dt.int8
    AF = mybir.ActivationFunctionType
    ALU = mybir.AluOpType
    DT_OUT = I8 if out_mode == "i8" else F16

    nc = bacc_mod.Bacc()
    aT_d = nc.dram_tensor("aT", [128, NT, 128], F16, kind="ExternalInput")
    lb_d = nc.dram_tensor("lb", [128, K2, 128], F16, kind="ExternalInput")
    qb_d = nc.dram_tensor("qb", [128, 1 + NT], FP32, kind="ExternalInput")
    out_d = nc.dram_tensor("out", [NSHARD, M], DT_OUT, kind="ExternalOutput")

    # lb chunks aligned to group boundaries (group g needs pairs 4g..4g+4):
    # small head chunks, then the big-descriptor rest
    CHUNKS = [(0, 4), (4, 12), (12, 32)]

    # evac engine per group, per tile (cadence: DVE ~1.25us, ACT ~1.09us per
    # 1024-elem evac). Uniform 4:4 keeps the per-tile slot pipeline smooth;
    # tile 0 gives ACT one extra (input gating leaves it slack during ramp).
    EMAP4 = [1, 1, 0, 0, 1, 1, 0, 0]    # 4 DVE evacs
    EMAP3 = [1, 1, 0, 0, 1, 0, 0, 0]    # 3 DVE evacs
    EMAP4t = [1, 1, 0, 0, 1, 0, 1, 0]   # 4 DVE, last two on both engines
    EMAPS = [EMAP3] + [EMAP4] * (NT - 2) + [EMAP4t]

    with tile.TileContext(nc) as tc, ExitStack() as ctx:
        apool = ctx.enter_context(tc.tile_pool(name="apool", bufs=1))
        bpool = ctx.enter_context(tc.tile_pool(name="bpool", bufs=len(CHUNKS)))
        mmps = ctx.enter_context(tc.tile_pool(name="mmps", bufs=1, space="PSUM"))
        stage = ctx.enter_context(tc.tile_pool(name="stage", bufs=NT))

        # warm the ACT function-table set (Identity) before any real work so
        # the ~2.7us ACT_TABLE_LOAD+drain overlaps the input DMA phase
        warm = apool.tile([128, 2], FP32)
        nc.scalar.activation(warm[:, 1:2], warm[:, 0:1], AF.Identity)

        # -------- input DMAs, all on the sync HWDGE ring (strict FIFO) -----
        # dummy first (absorbs SDMA engine wake-up latency: engine 15 starts
        # ~3us late and every completion waits all 16 engines' sem incs),
        # then tile-0's small pieces, then the big chunks, aT rest last.
        wdma = apool.tile([128, 1], FP32)
        nc.sync.dma_start(out=wdma, in_=qb_d[:, 0:1])
        aT0 = apool.tile([128, 1, 128], F16)
        nc.sync.dma_start(out=aT0, in_=aT_d[:, 0:1, :])
        lbs = []
        c0, c1 = CHUNKS[0]
        lb0 = bpool.tile([128, c1 - c0, 128], F16, tag="lb")
        lbs.append(lb0)
        nc.sync.dma_start(out=lb0, in_=lb_d[:, c0:c1, :])
        qb = apool.tile([128, 1 + NT], FP32)
        nc.sync.dma_start(out=qb, in_=qb_d[:, :])
        c0, c1 = CHUNKS[1]
        lbc = bpool.tile([128, c1 - c0, 128], F16, tag="lb")
        lbs.append(lbc)
        nc.sync.dma_start(out=lbc, in_=lb_d[:, c0:c1, :])
        # aT rest BEFORE the final chunk: tile 1's early groups then release
        # before tile 0's c2-gated ones and can fill the PE's wait gap
        aTr = apool.tile([128, NT - 1, 128], F16)
        nc.sync.dma_start(out=aTr, in_=aT_d[:, 1:NT, :])
        c0, c1 = CHUNKS[2]
        lbc = bpool.tile([128, c1 - c0, 128], F16, tag="lb")
        lbs.append(lbc)
        nc.sync.dma_start(out=lbc, in_=lb_d[:, c0:c1, :])

        # per-engine copies of qb: evacs then depend only on the PE sem plus
        # same-engine program order (one wait -> no per-evac event semaphore)
        qbv = apool.tile([128, 1 + NT], FP32)
        nc.vector.tensor_copy(qbv, qb)
        qba = apool.tile([128, 1 + NT], FP32)
        nc.scalar.copy(qba, qb)

        def rhs(g, half):
            for (c0, c1), lbc in zip(CHUNKS, lbs):
                if c0 <= 4 * g and 4 * g + 4 <= c1:
                    lo = 4 * g - c0
                    return lbc[half * 64 : half * 64 + 64, lo : lo + 4, :]
            raise AssertionError

        # ---------------- main GEMM + fused evac ----------------
        # psum: all 8 banks as one tile; group g -> slot s=g%4 (banks 2s,2s+1)
        ps = mmps.tile([128, 8, 512], FP32)
        # out HBM: (t p) (h g i) with h=2 halves, g=8 groups, i=512
        out_r = out_d[:, :].rearrange(
            "(t p) (h g i) -> t p h g i", p=128, h=2, i=512
        )
        # emission order: interleave tile 1's early-chunk groups (released by
        # aTr+c0+c1) into tile 0's wait for the last lb chunk, so the
        # in-order PE queue never stalls on c2. psum slot = emission idx % 4.
        seq = (
            [(0, g) for g in range(3)]
            + [(1, g) for g in range(3)]
            + [(2, g) for g in range(3)]
            + [(0, g) for g in range(3, G)]
            + [(1, g) for g in range(3, G)]
            + [(2, g) for g in range(3, G)]
            + [(t, g) for t in range(3, NT) for g in range(G)]
        )
        sbs = {}
        for i, (t, g) in enumerate(seq):
            if True:
                if t not in sbs:
                    out_sb = stage.tile([128, 2, 8, 512], DT_OUT, tag="out_sb")
                    sbs[t] = out_sb
                out_sb = sbs[t]
                lhsT = aT0[:, 0, :] if t == 0 else aTr[:, t - 1, :]
                s = i % 4
                # h0: partitions 0-63 (m = [512g, 512g+512)),
                # h1: partitions 64-127 (m = [4096+512g, ...))
                nc.tensor.matmul(
                    ps[:, 2 * s], lhsT[0:64, :], rhs(g, 0), start=True, stop=True
                )
                nc.tensor.matmul(
                    ps[:, 2 * s + 1],
                    lhsT[64:128, :],
                    rhs(g, 1),
                    start=True,
                    stop=True,
                )
                # fused affine+quantize evac, FD=1024 (one group, 2 banks)
                dst = out_sb[:, :, g, :]
                src = ps[:, 2 * s : 2 * s + 2, :]
                if EMAPS[t][g]:
                    nc.vector.tensor_scalar(
                        dst, src, qbv[:, 0:1], qbv[:, 1 + t : 2 + t],
                        ALU.mult, ALU.add,
                    )
                else:
                    nc.scalar.activation(
                        dst, src, AF.Identity,
                        bias=qba[:, 1 + t : 2 + t], scale=qba[:, 0:1],
                    )
                # stores every 4 groups; last tile drains in finer pieces
                step = 1 if t == NT - 1 else 4
                if g % step == step - 1:
                    j = g - step + 1
                    nc.sync.dma_start(
                        out=out_r[t, :, :, j : j + step, :],
                        in_=out_sb[:, :, j : j + step, :],
                    )
    nc.compile()
    return nc


def _prep(a, b, out_mode):
    """Host-side operand prep: log(b)^T, entropy/bias, layouts, quant range."""
    a32 = np.ascontiguousarray(np.asarray(a, dtype=np.float32))
    b32 = np.ascontiguousarray(np.asarray(b, dtype=np.float32))

    la_h = np.log(a32)                      # (N, D)
    lb_h = np.log(b32)                      # (M, D)
    ent_h = np.einsum("nd,nd->n", a32, la_h)

    if out_mode == "i8":
        # rigorous bounds on out = ent[n] - a[n].lb[m]  (a >= 0)
        colmax = lb_h.max(axis=0)           # (D,)
        colmin = lb_h.min(axis=0)
        lo = float((ent_h - a32 @ colmax).min())
        hi_cs = ent_h + np.linalg.norm(a32, axis=1) * float(
            np.linalg.norm(lb_h, axis=1).max()
        )
        hi_cm = ent_h - a32 @ colmin
        hi = float(np.minimum(hi_cs, hi_cm).max())
        pad = 0.2 + 1e-3 * (hi - lo)        # fp16 GEMM drift margin
        lo -= pad
        hi += pad
        inv = 253.0 / (hi - lo)
        cst = -lo * inv - 126.5
    else:
        inv, lo, cst = 1.0, 0.0, 0.0

    # lb[q, k, j] = log b[(k + 32*(q>=64))*128 + j, q%64]
    lbT = (
        lb_h.reshape(2, 32, 128, D)
        .transpose(0, 3, 1, 2)
        .reshape(128, K2, 128)
        .astype(np.float16)
    )
    lbT = np.ascontiguousarray(lbT)

    in_maps = []
    for i in range(NCORES):
        A = a32[i * NSHARD : (i + 1) * NSHARD]
        # aT[q, t, j] = -A[t*128 + j, q%64], duplicated across halves
        aTh = (-A).reshape(NT, 128, D).transpose(2, 0, 1).astype(np.float16)
        aT = np.ascontiguousarray(np.concatenate([aTh, aTh], axis=0))
        # qb: [inv | B], B[p, t] = ent[t*128+p]*inv + cst
        qb = np.empty((128, 1 + NT), np.float32)
        qb[:, 0] = inv
        ent_i = ent_h[i * NSHARD : (i + 1) * NSHARD].reshape(NT, 128).T
        qb[:, 1:] = ent_i * inv + cst
        in_maps.append({"aT": aT, "lb": lbT, "qb": qb})
    return in_maps, inv, lo


def _run(a, b, trace=False):
    from concourse.bass_utils import run_bass_kernel_spmd

    if OUT_MODE not in _CACHE:
        _CACHE[OUT_MODE] = _build(OUT_MODE)
    nc = _CACHE[OUT_MODE]
    in_maps, inv, lo = _prep(a, b, OUT_MODE)
    res = run_bass_kernel_spmd(nc, in_maps, list(range(NCORES)), trace=trace)
    outs = []
    for r in res.results:
        q = np.asarray(r["out"])
        if OUT_MODE == "i8":
            outs.append((q.astype(np.float32) + 126.5) / inv + lo)
        else:
            outs.append(q.astype(np.float32))
    return np.concatenate(outs, axis=0), res


def kernel(a, b):
    out, _ = _run(a, b, trace=False)
    return out


# revision 59
# speedup vs baseline: 1.0160x; 1.0160x over previous
"""Pairwise KL divergence kernel for Trainium2, SPMD across 8 NeuronCores.

out[n, m] = sum_d a[n,d]*(log a[n,d] - log b[m,d])
          = ent[n] - (a @ log(b)^T)[n, m],  ent = rowsum(a * log a)

Sharding: a (and output rows) split 8 ways; b replicated.
Per core: a_shard (1024, 64), b (8192, 64) -> out_shard (1024, 8192).

Design (v3): the device does the O(N*M*D) GEMM and the O(N*M)
quantize+store — everything O((N+M)*D) is host-side operand prep
(log(b), entropy/bias, layout, fp16 cast), per the sharding hint
("a local GEMM against log(b)^T").

  - lb: log(b)^T fp16, m-tiles 0..31 on partitions 0..63 (d axis),
    m-tiles 32..63 on partitions 64..127, so the two K=64 matmuls of a
    group run CONCURRENTLY on disjoint PE row halves (full PE rate).
  - aT: -a_shard^T fp16 duplicated on both partition halves (negation
    makes psum = -cross so the evac is a pure mult+add).
  - qb: [inv | B] fp32; B[p,t] = ent*inv + cst encodes the entropy term
    and the int8 affine code in the evac's per-partition scalars.
  - main loop: 8 n-tiles x 8 groups; group g -> psum slot g%4 (2 banks);
    fused evac (tensor_scalar / activation: q = rne(psum*inv + B), int8,
    FD=1024) on a statically-assigned engine (~31:33 DVE:ACT balance;
    static slot->engine keeps each matmul's psum-free wait on one
    semaphore); stores every 4 groups, finer on the last tile.
  - int8 output (saturating RNE on HW) halves store traffic; the quant
    range [lo, hi] is a rigorous host bound (Cauchy-Schwarz + column
    extrema), so clipping never triggers; ~0.4% rel err vs the 2e-2 gate.
  - Evac is the roofline: PSUM fp32 reads run at 1 elem/cycle/partition
    (DVE 0.96 GHz + ACT 1.2 GHz), ~42 us/core for the 8.4M elems.
"""

import numpy as np

N, M, D = 8192, 8192, 64
NCORES = 8
NSHARD = N // NCORES          # 1024 rows of a per core
NT = NSHARD // 128            # 8 n-tiles per core
K2 = M // 128 // 2            # 32 m-tile pairs (h0: tiles 0..31, h1: 32..63)
G = 8                         # groups per n-tile; each = 1024 out cols

OUT_MODE = "i8"               # "i8" | "f16"

_CACHE = {}


def _build(out_mode):
    from contextlib import ExitStack

    import concourse.bacc as bacc_mod
    import concourse.bass as bass
    import concourse.mybir as mybir
    import concourse.tile as tile

    FP32 = mybir.dt.float32
    F16 = mybir.dt.float16
    I8 = mybir.dt.int8
    AF = mybir.ActivationFunctionType
    ALU = mybir.AluOpType
    DT_OUT = I8 if out_mode == "i8" else F16

    nc = bacc_mod.Bacc()
    aT_d = nc.dram_tensor("aT", [128, NT, 128], F16, kind="ExternalInput")
    lb_d = nc.dram_tensor("lb", [128, K2, 128], F16, kind="ExternalInput")
    qb_d = nc.dram_tensor("qb", [128, 1 + NT], FP32, kind="ExternalInput")
    out_d = nc.dram_tensor("out", [NSHARD, M], DT_OUT, kind="ExternalOutput")

    # lb chunks aligned to group boundaries (group g needs pairs 4g..4g+4):
    # small head chunks, then the big-descriptor rest
    CHUNKS = [(0, 4), (4, 12), (12, 32)]

    # evac engine per group, per tile (cadence: DVE ~1.25us, ACT ~1.09us per
    # 1024-elem evac). Uniform 4:4 keeps the per-tile slot pipeline smooth;
    # tile 0 gives ACT one extra (input gating leaves it slack during ramp).
    EMAP4 = [1, 1, 0, 0, 1, 1, 0, 0]    # 4 DVE evacs
    EMAP3 = [1, 1, 0, 0, 1, 0, 0, 0]    # 3 DVE evacs
    EMAP4t = [1, 1, 0, 0, 1, 0, 1, 0]   # 4 DVE, last two on both engines
    EMAPS = [EMAP3] + [EMAP4] * (NT - 2) + [EMAP4t]

    with tile.TileContext(nc) as tc, ExitStack() as ctx:
        apool = ctx.enter_context(tc.tile_pool(name="apool", bufs=1))
        bpool = ctx.enter_context(tc.tile_pool(name="bpool", bufs=len(CHUNKS)))
        mmps = ctx.enter_context(tc.tile_pool(name="mmps", bufs=1, space="PSUM"))
        stage = ctx.enter_context(tc.tile_pool(name="stage", bufs=NT))

        # warm the ACT function-table set (Identity) before any real work so
        # the ~2.7us ACT_TABLE_LOAD+drain overlaps the input DMA phase
        warm = apool.tile([128, 2], FP32)
        nc.scalar.activation(warm[:, 1:2], warm[:, 0:1], AF.Identity)

        # -------- input DMAs, all on the sync HWDGE ring (strict FIFO) -----
        # dummy first (absorbs SDMA engine wake-up latency: engine 15 starts
        # ~3us late and every completion waits all 16 engines' sem incs),
        # then tile-0's small pieces, then the big chunks, aT rest last.
        wdma = apool.tile([128, 1], FP32)
        nc.sync.dma_start(out=wdma, in_=qb_d[:, 0:1])
        aT0 = apool.tile([128, 1, 128], F16)
        nc.sync.dma_start(out=aT0, in_=aT_d[:, 0:1, :])
        lbs = []
        c0, c1 = CHUNKS[0]
        lb0 = bpool.tile([128, c1 - c0, 128], F16, tag="lb")
        lbs.append(lb0)
        nc.sync.dma_start(out=lb0, in_=lb_d[:, c0:c1, :])
        qb = apool.tile([128, 1 + NT], FP32)
        nc.sync.dma_start(out=qb, in_=qb_d[:, :])
        c0, c1 = CHUNKS[1]
        lbc = bpool.tile([128, c1 - c0, 128], F16, tag="lb")
        lbs.append(lbc)
        nc.sync.dma_start(out=lbc, in_=lb_d[:, c0:c1, :])
        # aT rest BEFORE the final chunk: tile 1's early groups then release
        # before tile 0's c2-gated ones and can fill the PE's wait gap
        aTr = apool.tile([128, NT - 1, 128], F16)
        nc.sync.dma_start(out=aTr, in_=aT_d[:, 1:NT, :])
        c0, c1 = CHUNKS[2]
        lbc = bpool.tile([128, c1 - c0, 128], F16, tag="lb")
        lbs.append(lbc)
        nc.sync.dma_start(out=lbc, in_=lb_d[:, c0:c1, :])

        # per-engine copies of qb: evacs then depend only on the PE sem plus
        # same-engine program order (one wait -> no per-evac event semaphore)
        qbv = apool.tile([128, 1 + NT], FP32)
        nc.vector.tensor_copy(qbv, qb)
        qba = apool.tile([128, 1 + NT], FP32)
        nc.scalar.copy(qba, qb)

        def rhs(g, half):
            for (c0, c1), lbc in zip(CHUNKS, lbs):
                if c0 <= 4 * g and 4 * g + 4 <= c1:
                    lo = 4 * g - c0
                    return lbc[half * 64 : half * 64 + 64, lo : lo + 4, :]
            raise AssertionError

        # ---------------- main GEMM + fused evac ----------------
        # psum: all 8 banks as one tile; group g -> slot s=g%4 (banks 2s,2s+1)
        ps = mmps.tile([128, 8, 512], FP32)
        # out HBM: (t p) (h g i) with h=2 halves, g=8 groups, i=512
        out_r = out_d[:, :].rearrange(
            "(t p) (h g i) -> t p h g i", p=128, h=2, i=512
        )
        # emission order: interleave tile 1's early-chunk groups (released by
        # aTr+c0+c1) into tile 0's wait for the last lb chunk, so the
        # in-order PE queue never stalls on c2. psum slot = emission idx % 4.
        seq = (
            [(0, g) for g in range(3)]
            + [(1, g) for g in range(3)]
            + [(0, g) for g in range(3, G)]
            + [(1, g) for g in range(3, G)]
            + [(t, g) for t in range(2, NT) for g in range(G)]
        )
        sbs = {}
        for i, (t, g) in enumerate(seq):
            if True:
                if t not in sbs:
                    out_sb = stage.tile([128, 2, 8, 512], DT_OUT, tag="out_sb")
                    sbs[t] = out_sb
                out_sb = sbs[t]
                lhsT = aT0[:, 0, :] if t == 0 else aTr[:, t - 1, :]
                s = i % 4
                # h0: partitions 0-63 (m = [512g, 512g+512)),
                # h1: partitions 64-127 (m = [4096+512g, ...))
                nc.tensor.matmul(
                    ps[:, 2 * s], lhsT[0:64, :], rhs(g, 0), start=True, stop=True
                )
                nc.tensor.matmul(
                    ps[:, 2 * s + 1],
                    lhsT[64:128, :],
                    rhs(g, 1),
                    start=True,
                    stop=True,
                )
                # fused affine+quantize evac, FD=1024 (one group, 2 banks)
                dst = out_sb[:, :, g, :]
                src = ps[:, 2 * s : 2 * s + 2, :]
                if EMAPS[t][g]:
                    nc.vector.tensor_scalar(
                        dst, src, qbv[:, 0:1], qbv[:, 1 + t : 2 + t],
                        ALU.mult, ALU.add,
                    )
                else:
                    nc.scalar.activation(
                        dst, src, AF.Identity,
                        bias=qba[:, 1 + t : 2 + t], scale=qba[:, 0:1],
                    )
                # stores every 4 groups; last tile drains in finer pieces
                step = 1 if t == NT - 1 else 4
                if g % step == step - 1:
                    j = g - step + 1
                    nc.sync.dma_start(
                        out=out_r[t, :, :, j : j + step, :],
                        in_=out_sb[:, :, j : j + step, :],
                    )
    nc.compile()
    return nc


def _prep(a, b, out_mode):
    """Host-side operand prep: log(b)^T, entropy/bias, layouts, quant range."""
    a32 = np.ascontiguousarray(np.asarray(a, dtype=np.float32))
    b32 = np.ascontiguousarray(np.asarray(b, dtype=np.float32))

    la_h = np.log(a32)                      # (N, D)
    lb_h = np.log(b32)                      # (M, D)
    ent_h = np.einsum("nd,nd->n", a32, la_h)

    if out_mode == "i8":
        # rigorous bounds on out = ent[n] - a[n].lb[m]  (a >= 0)
        colmax = lb_h.max(axis=0)           # (D,)
        colmin = lb_h.min(axis=0)
        lo = float((ent_h - a32 @ colmax).min())
        hi_cs = ent_h + np.linalg.norm(a32, axis=1) * float(
            np.linalg.norm(lb_h, axis=1).max()
        )
        hi_cm = ent_h - a32 @ colmin
        hi = float(np.minimum(hi_cs, hi_cm).max())
        pad = 0.2 + 1e-3 * (hi - lo)        # fp16 GEMM drift margin
        lo -= pad
        hi += pad
        inv = 253.0 / (hi - lo)
        cst = -lo * inv - 126.5
    else:
        inv, lo, cst = 1.0, 0.0, 0.0

    # lb[q, k, j] = log b[(k + 32*(q>=64))*128 + j, q%64]
    lbT = (
        lb_h.reshape(2, 32, 128, D)
        .transpose(0, 3, 1, 2)
        .reshape(128, K2, 128)
        .astype(np.float16)
    )
    lbT = np.ascontiguousarray(lbT)

    in_maps = []
    for i in range(NCORES):
        A = a32[i * NSHARD : (i + 1) * NSHARD]
        # aT[q, t, j] = -A[t*128 + j, q%64], duplicated across halves
        aTh = (-A).reshape(NT, 128, D).transpose(2, 0, 1).astype(np.float16)
        aT = np.ascontiguousarray(np.concatenate([aTh, aTh], axis=0))
        # qb: [inv | B], B[p, t] = ent[t*128+p]*inv + cst
        qb = np.empty((128, 1 + NT), np.float32)
        qb[:, 0] = inv
        ent_i = ent_h[i * NSHARD : (i + 1) * NSHARD].reshape(NT, 128).T
        qb[:, 1:] = ent_i * inv + cst
        in_maps.append({"aT": aT, "lb": lbT, "qb": qb})
    return in_maps, inv, lo


def _run(a, b, trace=False):
    from concourse.bass_utils import run_bass_kernel_spmd

    if OUT_MODE not in _CACHE:
        _CACHE[OUT_MODE] = _build(OUT_MODE)
    nc = _CACHE[OUT_MODE]
    in_maps, inv, lo = _prep(a, b, OUT_MODE)
    res = run_bass_kernel_spmd(nc, in_maps, list(range(NCORES)), trace=trace)
    outs = []
    for r in res.results:
        q = np.asarray(r["out"])
        if OUT_MODE == "i8":
            outs.append((q.astype(np.float32) + 126.5) / inv + lo)
        else:
            outs.append(q.astype(np.float32))
    return np.concatenate(outs, axis=0), res


def kernel(a, b):
    out, _ = _run(a, b, trace=False)
    return out
